# revision 1
# baseline (speedup 1.0000x reference)
"""Trainium2 Bass kernel for the 2-layer GCN (nn_CGNN_70566312673786).

Strategy (8 NeuronCores, SPMD):
  - Nodes (rows of x / segment_sum outputs) are sharded 8 ways; edges are
    partitioned by destination row and sorted/padded by (128-row block,
    int16 col bucket) on the host (index prep only - all float math runs
    on device).
  - segment_sum is computed as one-hot matmuls on the PE: for each
    128-edge tile, a [128e x 128r] selection matrix O (O[e,r] = C_e if
    row_e == r) is built with one fused DVE tensor_scalar op, and
    PSUM accumulates O.T @ gathered_features.
  - x[col] source-node features are fetched with dma_gather (SWDGE),
    one 640-row gather per (block, bucket).
  - deg^-1/2 source-side scaling is pre-folded into the gathered table
    (xs = dis * x); the destination side is applied per-partition after
    accumulation.  Layer-2 propagates hw2s = dis * (h @ W2.T) (40->64
    padded) so its gather moves 64-wide rows.
  - 3 NEFF launches with host-side concatenation (pure data movement)
    between them: (1) deg/dis/xs, (2) layer-1 -> hw2s slices,
    (3) layer-2 -> output slices.
"""

import contextlib

import numpy as np

import concourse.bacc as bacc
import concourse.bass as bass
import concourse.mybir as mybir
import concourse.tile as tile
from concourse.bass_utils import run_bass_kernel_spmd

# ---- problem constants (hardcoded per the task contract) ----
N = 100000
E = 1600000
D = 128            # feature dim
H = 128            # hidden dim
C_OUT = 40         # output classes
C_PAD = 64         # padded output width (dma_gather needs 256B rows)

NCORES = 8
RPC = 12544        # rows per core (8 * 12544 = 100352 >= N)
NPAD = NCORES * RPC
NBLK = RPC // 128  # 98 row blocks per core
NBUCK = 4
BUCK = NPAD // NBUCK  # 25088 rows per int16 col bucket

F32 = mybir.dt.float32
I16 = mybir.dt.int16
I32 = mybir.dt.int32

IOTA128 = np.tile(np.arange(128, dtype=np.float32), (128, 1))
IDENT128 = np.eye(128, dtype=np.float32)


def _wrap_idx(flat):
    """dma_gather index layout: idx i -> [i % 16, i // 16], replicated to
    128 partitions."""
    n = flat.shape[0]
    w = flat.reshape(n // 16, 16).T  # [16, n//16]
    return np.tile(w, (8, 1))


def _build_host_layouts(row, col, cv):
    """Sort edges by destination row; build per-core padded layouts.

    Layer-1 (deg pass): edges grouped by 128-row block, T1 tiles/block.
    Layer-2/3 (spmm): edges grouped by (block, int16-bucket), padded to
    TBB 128-edge tiles with -1 indices (skipped by dma_gather) and a
    per-group valid count.
    """
    order = np.argsort(row, kind="stable")
    rs = row[order].astype(np.int64)
    cs = col[order].astype(np.int64)
    ws = cv[order].astype(np.float32)

    core_of = rs // RPC
    per_core = []
    T1 = 1
    TBB = 2
    for c in range(NCORES):
        m = core_of == c
        r_loc = (rs[m] - c * RPC).astype(np.int64)
        cc = cs[m]
        ww = ws[m]
        blk = r_loc // 128
        rl = (r_loc % 128).astype(np.float32)
        bk = cc // BUCK
        per_core.append((blk, rl, bk, cc, ww))
        cnt_b = np.bincount(blk, minlength=NBLK)
        if cnt_b.size:
            T1 = max(T1, int(np.ceil(cnt_b.max() / 128)))
        cnt_bb = np.bincount(blk * NBUCK + bk, minlength=NBLK * NBUCK)
        if cnt_bb.size:
            TBB = max(TBB, int(np.ceil(cnt_bb.max() / 128)))

    NQ = NBLK * NBUCK
    l1_row = np.zeros((NCORES, NBLK * T1, 128), np.float32)
    l1_c = np.zeros((NCORES, NBLK * T1, 128), np.float32)
    l2_row = np.zeros((NCORES, NQ * TBB, 128), np.float32)
    l2_c = np.zeros((NCORES, NQ * TBB, 128), np.float32)
    l2_idx = np.zeros((NCORES, NQ, TBB * 128), np.int16)
    l2_cnt = np.ones((NCORES, NQ), np.int32)

    for c in range(NCORES):
        blk, rl, bk, cc, ww = per_core[c]
        # launch-1 layout
        o = np.argsort(blk, kind="stable")
        blk1, rl1, ww1 = blk[o], rl[o], ww[o]
        starts = np.searchsorted(blk1, np.arange(NBLK))
        ends = np.searchsorted(blk1, np.arange(NBLK), side="right")
        for b in range(NBLK):
            s, e = starts[b], ends[b]
            n = e - s
            if n == 0:
                continue
            l1_row[c, b * T1:(b + 1) * T1].reshape(-1)[:n] = rl1[s:e]
            l1_c[c, b * T1:(b + 1) * T1].reshape(-1)[:n] = ww1[s:e]
        # layer-2/3 layout
        key = blk * NBUCK + bk
        o = np.argsort(key, kind="stable")
        key2, rl2, cc2, ww2 = key[o], rl[o], cc[o], ww[o]
        starts = np.searchsorted(key2, np.arange(NQ))
        ends = np.searchsorted(key2, np.arange(NQ), side="right")
        for q in range(NQ):
            s, e = starts[q], ends[q]
            n = e - s
            l2_row[c, q * TBB:(q + 1) * TBB].reshape(-1)[:n] = rl2[s:e]
            l2_c[c, q * TBB:(q + 1) * TBB].reshape(-1)[:n] = ww2[s:e]
            if n > 0:
                l2_idx[c, q, :n] = (cc2[s:e] % BUCK).astype(np.int16)
                l2_cnt[c, q] = n
            else:
                # keep one valid dg index (C=0 slot) so HW/sim are happy
                l2_idx[c, q, 0] = 0
                l2_cnt[c, q] = 1

    # SBUF layouts: [128, ntiles]
    l1_row = np.ascontiguousarray(l1_row.transpose(0, 2, 1))
    l1_c = np.ascontiguousarray(l1_c.transpose(0, 2, 1))
    l2_row = np.ascontiguousarray(l2_row.transpose(0, 2, 1))
    l2_c = np.ascontiguousarray(l2_c.transpose(0, 2, 1))
    l2_idx_w = np.zeros((NCORES, 128, NQ * TBB * 8), np.int16)
    W8 = TBB * 8
    for c in range(NCORES):
        for q in range(NQ):
            l2_idx_w[c, :, q * W8:(q + 1) * W8] = _wrap_idx(l2_idx[c, q])
    return (l1_row, l1_c, l2_row, l2_c, l2_idx_w, l2_cnt, T1, TBB)


def _build_launch1(T1, reps=1):
    """deg (one-hot matmuls) -> dis -> xs, all row-local per core."""
    nc = bacc.Bacc("TRN2", target_bir_lowering=False)
    x_sl = nc.dram_tensor("x_sl", [RPC, D], F32, kind="ExternalInput")
    rowt = nc.dram_tensor("rowt", [128, NBLK * T1], F32, kind="ExternalInput")
    ct = nc.dram_tensor("ct", [128, NBLK * T1], F32, kind="ExternalInput")
    iota = nc.dram_tensor("iota", [128, 128], F32, kind="ExternalInput")
    xs_sl = nc.dram_tensor("xs_sl", [RPC, D], F32, kind="ExternalOutput")
    dis_sl = nc.dram_tensor("dis_sl", [128, NBLK], F32, kind="ExternalOutput")

    with tile.TileContext(nc) as tc:
        with tc.tile_pool(name="const", bufs=1) as cpool, \
             tc.tile_pool(name="work", bufs=3) as wpool, \
             tc.tile_pool(name="small", bufs=4) as spool, \
             tc.tile_pool(name="psum", bufs=2, space="PSUM") as ppool:
            rl = cpool.tile([128, NBLK * T1], F32)
            cw = cpool.tile([128, NBLK * T1], F32)
            io = cpool.tile([128, 128], F32)
            dis_all = cpool.tile([128, NBLK], F32)
            nc.sync.dma_start(out=rl[:], in_=rowt[:, :])
            nc.sync.dma_start(out=cw[:], in_=ct[:, :])
            nc.sync.dma_start(out=io[:], in_=iota[:, :])
            rep = tc.For_i(0, reps, 1) if reps > 1 else contextlib.nullcontext()
            with rep:
              for b in range(NBLK):
                deg_ps = ppool.tile([128, 1], F32, tag="deg")
                for t in range(T1):
                    k = b * T1 + t
                    oh = wpool.tile([128, 128], F32, tag="oh")
                    nc.vector.tensor_scalar(
                        out=oh[:], in0=io[:],
                        scalar1=rl[:, k:k + 1], scalar2=None,
                        op0=mybir.AluOpType.is_equal,
                    )
                    nc.tensor.matmul(
                        out=deg_ps[:], lhsT=oh[:], rhs=cw[:, k:k + 1],
                        start=(t == 0), stop=(t == T1 - 1),
                    )
                degs = spool.tile([128, 1], F32, tag="degs")
                z = spool.tile([128, 1], F32, tag="z")
                sq = spool.tile([128, 1], F32, tag="sq")
                rec = spool.tile([128, 1], F32, tag="rec")
                nc.vector.tensor_copy(out=degs[:], in_=deg_ps[:])
                nc.vector.tensor_scalar(
                    out=z[:], in0=degs[:], scalar1=0.0, scalar2=None,
                    op0=mybir.AluOpType.is_le)
                nc.vector.tensor_tensor(
                    out=degs[:], in0=degs[:], in1=z[:],
                    op=mybir.AluOpType.add)
                nc.scalar.sqrt(out=sq[:], in_=degs[:])
                nc.vector.reciprocal(out=rec[:], in_=sq[:])
                nc.vector.tensor_scalar(
                    out=z[:], in0=z[:], scalar1=-1.0, scalar2=1.0,
                    op0=mybir.AluOpType.mult, op1=mybir.AluOpType.add)
                nc.vector.tensor_tensor(
                    out=dis_all[:, b:b + 1], in0=rec[:], in1=z[:],
                    op=mybir.AluOpType.mult)
                xt = wpool.tile([128, D], F32, tag="xt")
                nc.sync.dma_start(out=xt[:], in_=x_sl[b * 128:(b + 1) * 128, :])
                xst = wpool.tile([128, D], F32, tag="xst")
                nc.vector.tensor_scalar(
                    out=xst[:], in0=xt[:], scalar1=dis_all[:, b:b + 1],
                    scalar2=None, op0=mybir.AluOpType.mult)
                nc.sync.dma_start(out=xs_sl[b * 128:(b + 1) * 128, :],
                                  in_=xst[:])
            nc.sync.dma_start(out=dis_sl[:, :], in_=dis_all[:])
    nc.compile()
    return nc


def _build_spmm_launch(TBB, layer, reps=1):
    """layer 1: gather xs (128 wide) -> spmm -> linear1+relu -> linear2
       -> hw2s slice.  layer 2: gather hw2s (64 wide) -> spmm -> +b2
       -> out slice."""
    W_IN = D if layer == 1 else C_PAD
    NQ = NBLK * NBUCK
    W8 = TBB * 8
    nc = bacc.Bacc("TRN2", target_bir_lowering=False)
    tab = nc.dram_tensor("tab", [NPAD, W_IN], F32, kind="ExternalInput")
    rowt = nc.dram_tensor("rowt", [128, NQ * TBB], F32, kind="ExternalInput")
    ct = nc.dram_tensor("ct", [128, NQ * TBB], F32, kind="ExternalInput")
    idxt = nc.dram_tensor("idxt", [128, NQ * W8], I16, kind="ExternalInput")
    cntt = nc.dram_tensor("cntt", [1, NQ], I32, kind="ExternalInput")
    iota = nc.dram_tensor("iota", [128, 128], F32, kind="ExternalInput")
    dis_sl = nc.dram_tensor("dis_sl", [128, NBLK], F32, kind="ExternalInput")
    if layer == 1:
        ident = nc.dram_tensor("ident", [128, 128], F32, kind="ExternalInput")
        w1t = nc.dram_tensor("w1t", [D, H], F32, kind="ExternalInput")
        b1 = nc.dram_tensor("b1", [H, 1], F32, kind="ExternalInput")
        w2t = nc.dram_tensor("w2t", [H, C_PAD], F32, kind="ExternalInput")
    else:
        b2bc = nc.dram_tensor("b2bc", [128, C_PAD], F32, kind="ExternalInput")
    out_sl = nc.dram_tensor("out_sl", [RPC, C_PAD], F32, kind="ExternalOutput")

    with tile.TileContext(nc) as tc:
        with tc.tile_pool(name="const", bufs=1) as cpool, \
             tc.tile_pool(name="gat", bufs=3) as gpool, \
             tc.tile_pool(name="oh", bufs=4) as opool, \
             tc.tile_pool(name="tailA", bufs=2) as tpool, \
             tc.tile_pool(name="psum", bufs=2, space="PSUM") as ppool, \
             tc.tile_pool(name="psum2", bufs=2, space="PSUM") as ppool2, \
             contextlib.nullcontext():
            rl = cpool.tile([128, NQ * TBB], F32)
            cw = cpool.tile([128, NQ * TBB], F32)
            idxs = cpool.tile([128, NQ * W8], I16)
            cnts = cpool.tile([1, NQ], I32)
            io = cpool.tile([128, 128], F32)
            dis = cpool.tile([128, NBLK], F32)
            nc.sync.dma_start(out=rl[:], in_=rowt[:, :])
            nc.sync.dma_start(out=cw[:], in_=ct[:, :])
            nc.sync.dma_start(out=idxs[:], in_=idxt[:, :])
            nc.sync.dma_start(out=cnts[:], in_=cntt[:, :])
            nc.sync.dma_start(out=io[:], in_=iota[:, :])
            nc.sync.dma_start(out=dis[:], in_=dis_sl[:, :])
            if layer == 1:
                idn = cpool.tile([128, 128], F32)
                w1s = cpool.tile([D, H], F32)
                b1s = cpool.tile([H, 1], F32)
                w2s = cpool.tile([H, C_PAD], F32)
                nc.sync.dma_start(out=idn[:], in_=ident[:, :])
                nc.sync.dma_start(out=w1s[:], in_=w1t[:, :])
                nc.sync.dma_start(out=b1s[:], in_=b1[:, :])
                nc.sync.dma_start(out=w2s[:], in_=w2t[:, :])
            else:
                b2s = cpool.tile([128, C_PAD], F32)
                nc.sync.dma_start(out=b2s[:], in_=b2bc[:, :])

            rep = tc.For_i(0, reps, 1) if reps > 1 else contextlib.nullcontext()
            with rep:
              for b in range(NBLK):
                acc = ppool.tile([128, W_IN], F32, tag="acc")
                for k in range(NBUCK):
                    q = b * NBUCK + k
                    # dma_gather, all TBB tiles, -1 padded (pads emit
                    # no descriptors); valid count via register
                    g = gpool.tile([128, TBB * W_IN], F32, tag="g")
                    nc.gpsimd.dma_gather(
                        g[:].rearrange("p (t d) -> p t d", d=W_IN),
                        tab[k * BUCK:(k + 1) * BUCK, :],
                        idxs[:, q * W8:(q + 1) * W8],
                        TBB * 128, TBB * 128, W_IN,
                        single_packet=(TBB * 128 <= 1024),
                    )
                    for t in range(TBB):
                        kk = q * TBB + t
                        oh = opool.tile([128, 128], F32, tag="oh")
                        nc.vector.tensor_scalar(
                            out=oh[:], in0=io[:],
                            scalar1=rl[:, kk:kk + 1], scalar2=cw[:, kk:kk + 1],
                            op0=mybir.AluOpType.is_equal,
                            op1=mybir.AluOpType.mult,
                        )
                        nc.tensor.matmul(
                            out=acc[:], lhsT=oh[:],
                            rhs=g[:, t * W_IN:(t + 1) * W_IN],
                            start=(k == 0 and t == 0),
                            stop=(k == NBUCK - 1 and t == TBB - 1),
                        )
                if layer == 1:
                    s_sb = tpool.tile([128, D], F32, tag="s_sb")
                    nc.vector.tensor_scalar(
                        out=s_sb[:], in0=acc[:], scalar1=dis[:, b:b + 1],
                        scalar2=None, op0=mybir.AluOpType.mult)
                    st_ps = ppool2.tile([128, 128], F32, tag="st_ps")
                    nc.tensor.transpose(out=st_ps[:], in_=s_sb[:],
                                        identity=idn[:])
                    st_sb = tpool.tile([128, 128], F32, tag="st_sb")
                    nc.vector.tensor_copy(out=st_sb[:], in_=st_ps[:])
                    ht_ps = ppool2.tile([H, 128], F32, tag="ht_ps")
                    nc.tensor.matmul(out=ht_ps[:], lhsT=w1s[:], rhs=st_sb[:],
                                     start=True, stop=True)
                    ht_sb = tpool.tile([H, 128], F32, tag="ht_sb")
                    nc.scalar.activation(
                        out=ht_sb[:], in_=ht_ps[:],
                        func=mybir.ActivationFunctionType.Relu,
                        bias=b1s[:, 0:1], scale=1.0)
                    hw2_ps = ppool2.tile([128, C_PAD], F32, tag="hw2_ps")
                    nc.tensor.matmul(out=hw2_ps[:], lhsT=ht_sb[:], rhs=w2s[:],
                                     start=True, stop=True)
                    hw2_sb = tpool.tile([128, C_PAD], F32, tag="hw2_sb")
                    nc.vector.tensor_scalar(
                        out=hw2_sb[:], in0=hw2_ps[:], scalar1=dis[:, b:b + 1],
                        scalar2=None, op0=mybir.AluOpType.mult)
                    nc.sync.dma_start(
                        out=out_sl[b * 128:(b + 1) * 128, :], in_=hw2_sb[:])
                else:
                    o_sb = tpool.tile([128, C_PAD], F32, tag="o_sb")
                    nc.vector.tensor_scalar(
                        out=o_sb[:], in0=acc[:], scalar1=dis[:, b:b + 1],
                        scalar2=None, op0=mybir.AluOpType.mult)
                    nc.vector.tensor_tensor(
                        out=o_sb[:], in0=o_sb[:], in1=b2s[:],
                        op=mybir.AluOpType.add)
                    nc.sync.dma_start(
                        out=out_sl[b * 128:(b + 1) * 128, :], in_=o_sb[:])
    nc.compile()
    return nc


_CACHE = {}
LAST_HW_NS = None


def kernel(x, edge_index, C_values, W1, b1, W2, b2):
    x = np.asarray(x, np.float32)
    row = np.asarray(edge_index[0], np.int64)
    col = np.asarray(edge_index[1], np.int64)
    cv = np.asarray(C_values, np.float32)
    W1 = np.asarray(W1, np.float32)
    b1v = np.asarray(b1, np.float32)
    W2 = np.asarray(W2, np.float32)
    b2v = np.asarray(b2, np.float32)

    (l1r, l1c, l2r, l2c, l2i, l2cnt, T1, TBB) = \
        _build_host_layouts(row, col, cv)

    key = (T1, TBB)
    if key not in _CACHE:
        _CACHE[key] = (
            _build_launch1(T1),
            _build_spmm_launch(TBB, 1),
            _build_spmm_launch(TBB, 2),
        )
    nc1, nc2, nc3 = _CACHE[key]

    x_pad = np.zeros((NPAD, D), np.float32)
    x_pad[:N] = x

    cores = list(range(NCORES))
    in1 = [
        {"x_sl": x_pad[c * RPC:(c + 1) * RPC],
         "rowt": l1r[c], "ct": l1c[c], "iota": IOTA128}
        for c in cores
    ]
    r1 = run_bass_kernel_spmd(nc1, in1, core_ids=cores, trace=False)
    xs_full = np.concatenate([r1.results[c]["xs_sl"] for c in cores], axis=0)
    dis = [r1.results[c]["dis_sl"] for c in cores]

    w1t = np.ascontiguousarray(W1.T)                   # [D, H]
    w2t = np.zeros((H, C_PAD), np.float32)
    w2t[:, :C_OUT] = W2.T
    in2 = [
        {"tab": xs_full, "rowt": l2r[c], "ct": l2c[c], "idxt": l2i[c],
         "cntt": l2cnt[c].reshape(1, -1),
         "iota": IOTA128, "dis_sl": dis[c], "ident": IDENT128,
         "w1t": w1t, "b1": b1v.reshape(H, 1), "w2t": w2t}
        for c in cores
    ]
    r2 = run_bass_kernel_spmd(nc2, in2, core_ids=cores, trace=False)
    hw2s_full = np.concatenate([r2.results[c]["out_sl"] for c in cores],
                               axis=0)

    b2bc = np.zeros((128, C_PAD), np.float32)
    b2bc[:, :C_OUT] = b2v
    in3 = [
        {"tab": hw2s_full, "rowt": l2r[c], "ct": l2c[c], "idxt": l2i[c],
         "cntt": l2cnt[c].reshape(1, -1),
         "iota": IOTA128, "dis_sl": dis[c], "b2bc": b2bc}
        for c in cores
    ]
    r3 = run_bass_kernel_spmd(nc3, in3, core_ids=cores, trace=False)
    out = np.concatenate([r3.results[c]["out_sl"] for c in cores], axis=0)
    return np.ascontiguousarray(out[:N, :C_OUT])



# revision 4
# speedup vs baseline: 8655.5527x; 8655.5527x over previous
"""Trainium2 Bass kernel for the 2-layer GCN (nn_CGNN_70566312673786).

Strategy (8 NeuronCores, SPMD, 3 NEFF launches with host-side concat of
per-core slices between them — index prep / data movement only on host,
all float math on device):

  L1: deg = segment_sum(cv, row) computed WITHOUT one-hots: the host slots
      each edge's cv into a [128, NBLK, K1] per-dest-row layout and one DVE
      tensor_reduce produces deg for all rows; dis = rsqrt-guarded.
      xw1 = (dis * x) @ W1^T is emitted bf16 (spmm and the linear commute).
  L2: per-edge source rows xw1[col] are fetched with supergathers —
      one dma_gather per (8-block super, int16 col bucket), 4 SWDGE
      queues, >=8 gather buffers in flight (descriptor-rate bound
      otherwise).  segment_sum runs as one-hot matmuls on the PE in bf16:
      oh[e,r] = cv_e * (iota[r]==rl_e) built on DVE (fused
      is_equal+mult) / Pool / ACT (relu(cv - cv*(iota-rl)^2)) round-robin.
      Tail per 128-row block: h = relu(dis*acc + b1); hw2s = dis*(h@W2^T)
      written as [RPC, 128] bf16 rows (cols 0:64; 256B gather rows).
  L3: same supergather/spmm structure over hw2s; out = dis*acc + b2.
"""

import contextlib

import numpy as np

import concourse.bacc as bacc
import concourse.bass as bass
import concourse.mybir as mybir
import concourse.tile as tile
from concourse.bass_utils import run_bass_kernel_spmd

N = 100000
E = 1600000
D = 128
H = 128
C_OUT = 40
C_PAD = 64         # hw2 compute width (psum free dim)

NCORES = 8
RPC = 12544
NPAD = NCORES * RPC
NBLK = RPC // 128  # 98
NBUCK = 4
BUCK = NPAD // NBUCK  # 25088

SB = 8             # blocks per supergather
SUPERS = [SB] * (NBLK // SB) + ([NBLK % SB] if NBLK % SB else [])
NSWQ = 4           # swdge queues
GBUFS = 3          # gather buffers per (bucket,size) tag

F32 = mybir.dt.float32
BF16 = mybir.dt.bfloat16
I16 = mybir.dt.int16


def _np_bf16():
    import ml_dtypes
    return np.dtype(ml_dtypes.bfloat16)


def _wrap_idx(flat):
    """dma_gather idx layout: idx i -> [i % 16, i // 16] x8 replication."""
    n = flat.shape[0]
    w = flat.reshape(n // 16, 16).T
    return np.tile(w, (8, 1))


def build_host_layouts(row, col, cv):
    """Vectorized host index prep (no float math: slotting/sorting only)."""
    row = row.astype(np.int64)
    col = col.astype(np.int64)
    cv = cv.astype(np.float32)

    # deg layout: slot each edge per dest row
    order_r = np.argsort(row, kind="stable")
    rs = row[order_r]
    cvs = cv[order_r]
    starts = np.searchsorted(rs, np.arange(NPAD))
    slot = np.arange(E) - starts[rs]
    K1 = int(slot.max()) + 1
    cvrow = np.zeros((NCORES, 128, NBLK * K1), np.float32)
    cvrow[rs // RPC, rs % 128, ((rs % RPC) // 128) * K1 + slot] = cvs

    # spmm layout: group edges by (dest 128-block, col bucket)
    gblk = row // 128
    key = gblk * NBUCK + (col // BUCK)
    order = np.argsort(key, kind="stable")
    key_s = key[order]
    col_s = col[order]
    row_s = row[order]
    cv_s = cv[order]
    NG = NCORES * NBLK * NBUCK
    gstarts = np.searchsorted(key_s, np.arange(NG))
    gends = np.searchsorted(key_s, np.arange(NG), side="right")
    cnts = gends - gstarts
    TBB = int(np.ceil(cnts.max() / 128))
    NQ = NBLK * NBUCK

    pos = np.arange(E) - gstarts[key_s]
    gq = key_s % NQ
    gcore = key_s // NQ
    rl_flat = np.zeros((NCORES, NQ, TBB * 128), np.float32)
    cw_flat = np.zeros((NCORES, NQ, TBB * 128), np.float32)
    ix_flat = np.zeros((NCORES, NQ, TBB * 128), np.int16)  # 0-pads (see DGE)
    rl_flat[gcore, gq, pos] = (row_s % 128).astype(np.float32)
    cw_flat[gcore, gq, pos] = cv_s
    ix_flat[gcore, gq, pos] = (col_s % BUCK).astype(np.int16)

    rl_t = np.ascontiguousarray(
        rl_flat.reshape(NCORES, NQ * TBB, 128).transpose(0, 2, 1))
    cw_t = np.ascontiguousarray(
        cw_flat.reshape(NCORES, NQ * TBB, 128).transpose(0, 2, 1))

    # supergather idx streams ordered (super, bucket, block)
    idxw = []
    for c in range(NCORES):
        streams = []
        b0 = 0
        for sbs in SUPERS:
            for k in range(NBUCK):
                qs = [(b * NBUCK + k) for b in range(b0, b0 + sbs)]
                streams.append(ix_flat[c, qs].reshape(-1))
            b0 += sbs
        idxw.append(np.concatenate([_wrap_idx(s) for s in streams], axis=1))
    idxw = np.stack(idxw)

    return dict(k1=K1, cvrow=cvrow, rl=rl_t, cw=cw_t, idxw=idxw, tbb=TBB)


def make_consts():
    bf = _np_bf16()
    iota = np.tile(np.arange(128).astype(bf), (128, 1))
    ident = np.eye(128).astype(bf)
    return iota, ident


def build_launch1(K1, reps=1):
    nc = bacc.Bacc("TRN2", target_bir_lowering=False)
    x_sl = nc.dram_tensor("x_sl", [RPC, D], F32, kind="ExternalInput")
    cvrow = nc.dram_tensor("cvrow", [128, NBLK * K1], F32, kind="ExternalInput")
    w1t = nc.dram_tensor("w1t", [D, H], F32, kind="ExternalInput")
    identb = nc.dram_tensor("identb", [128, 128], BF16, kind="ExternalInput")
    xw1_sl = nc.dram_tensor("xw1_sl", [RPC, H], BF16, kind="ExternalOutput")
    dis_sl = nc.dram_tensor("dis_sl", [128, NBLK], F32, kind="ExternalOutput")

    with tile.TileContext(nc) as tc:
        with tc.tile_pool(name="const", bufs=1) as cpool, \
             tc.tile_pool(name="work", bufs=3) as wpool, \
             tc.tile_pool(name="psum", bufs=2, space="PSUM") as ppool, \
             tc.tile_pool(name="psum2", bufs=2, space="PSUM") as ppool2:
            cvr = cpool.tile([128, NBLK * K1], F32)
            w1s = cpool.tile([D, H], F32)
            w1b = cpool.tile([D, H], BF16)
            idb = cpool.tile([128, 128], BF16)
            deg = cpool.tile([128, NBLK], F32)
            z = cpool.tile([128, NBLK], F32)
            sq = cpool.tile([128, NBLK], F32)
            rec = cpool.tile([128, NBLK], F32)
            dis = cpool.tile([128, NBLK], F32)
            nc.sync.dma_start(out=cvr[:], in_=cvrow[:, :])
            nc.sync.dma_start(out=w1s[:], in_=w1t[:, :])
            nc.sync.dma_start(out=idb[:], in_=identb[:, :])
            nc.vector.tensor_copy(out=w1b[:], in_=w1s[:])

            rep = tc.For_i(0, reps, 1) if reps > 1 else contextlib.nullcontext()
            with rep:
                nc.vector.tensor_reduce(
                    out=deg[:], in_=cvr[:].rearrange("p (b k) -> p b k", k=K1),
                    axis=mybir.AxisListType.X, op=mybir.AluOpType.add)
                nc.vector.tensor_scalar(
                    out=z[:], in0=deg[:], scalar1=0.0, scalar2=None,
                    op0=mybir.AluOpType.is_le)
                nc.vector.tensor_tensor(
                    out=deg[:], in0=deg[:], in1=z[:], op=mybir.AluOpType.add)
                nc.scalar.sqrt(out=sq[:], in_=deg[:])
                nc.vector.reciprocal(out=rec[:], in_=sq[:])
                nc.vector.tensor_scalar(
                    out=z[:], in0=z[:], scalar1=-1.0, scalar2=1.0,
                    op0=mybir.AluOpType.mult, op1=mybir.AluOpType.add)
                nc.vector.tensor_tensor(
                    out=dis[:], in0=rec[:], in1=z[:], op=mybir.AluOpType.mult)
                nc.sync.dma_start(out=dis_sl[:, :], in_=dis[:])

                for b in range(NBLK):
                    xt = wpool.tile([128, D], F32, tag="xt")
                    nc.sync.dma_start(out=xt[:],
                                      in_=x_sl[b * 128:(b + 1) * 128, :])
                    xb = wpool.tile([128, D], BF16, tag="xb")
                    nc.vector.tensor_scalar(
                        out=xb[:], in0=xt[:], scalar1=dis[:, b:b + 1],
                        scalar2=None, op0=mybir.AluOpType.mult)
                    xbT_ps = ppool.tile([128, 128], BF16, tag="xbT")
                    nc.tensor.transpose(out=xbT_ps[:], in_=xb[:],
                                        identity=idb[:])
                    xbT = wpool.tile([128, 128], BF16, tag="xbTs")
                    nc.scalar.copy(out=xbT[:], in_=xbT_ps[:])
                    xw1_ps = ppool2.tile([128, H], F32, tag="xw1")
                    nc.tensor.matmul(out=xw1_ps[:], lhsT=xbT[:], rhs=w1b[:],
                                     start=True, stop=True)
                    xw1b = wpool.tile([128, H], BF16, tag="xw1b")
                    nc.scalar.copy(out=xw1b[:], in_=xw1_ps[:])
                    nc.sync.dma_start(out=xw1_sl[b * 128:(b + 1) * 128, :],
                                      in_=xw1b[:])
    nc.compile()
    return nc


def build_spmm(TBB, layer, reps=1,
               onehot_plan=(("v", 1),)):
    NQ = NBLK * NBUCK
    NQT = NQ * TBB
    NIDX16 = NQT * 8
    W_IN = 128
    MM_W = H if layer == 1 else C_PAD
    nc = bacc.Bacc("TRN2", target_bir_lowering=False, num_swdge_queues=NSWQ)
    tab = nc.dram_tensor("tab", [NPAD, W_IN], BF16, kind="ExternalInput")
    rowt = nc.dram_tensor("rowt", [128, NQT], F32, kind="ExternalInput")
    ct = nc.dram_tensor("ct", [128, NQT], F32, kind="ExternalInput")
    idxt = nc.dram_tensor("idxt", [128, NIDX16], I16, kind="ExternalInput")
    iotab = nc.dram_tensor("iotab", [128, 128], BF16, kind="ExternalInput")
    dis_sl = nc.dram_tensor("dis_sl", [128, NBLK], F32, kind="ExternalInput")
    if layer == 1:
        identb = nc.dram_tensor("identb", [128, 128], BF16,
                                kind="ExternalInput")
        b1bc = nc.dram_tensor("b1bc", [128, H], F32, kind="ExternalInput")
        w2t = nc.dram_tensor("w2t", [H, C_PAD], F32, kind="ExternalInput")
        out_sl = nc.dram_tensor("out_sl", [RPC, W_IN], BF16,
                                kind="ExternalOutput")
    else:
        b2bc = nc.dram_tensor("b2bc", [128, C_PAD], F32, kind="ExternalInput")
        out_sl = nc.dram_tensor("out_sl", [RPC, C_PAD], F32,
                                kind="ExternalOutput")

    eng_seq = []
    for e, w in onehot_plan:
        eng_seq += [e] * w

    with tile.TileContext(nc) as tc:
        with tc.tile_pool(name="const", bufs=1) as cpool, \
             tc.tile_pool(name="gat", bufs=GBUFS) as gpool, \
             tc.tile_pool(name="oh", bufs=6) as opool, \
             tc.tile_pool(name="tail", bufs=3) as tpool, \
             tc.tile_pool(name="acc", bufs=2, space="PSUM") as apool, \
             tc.tile_pool(name="ps2", bufs=2, space="PSUM") as ppool2:
            use_act = any(e == "a" for e, _ in onehot_plan)
            rl = cpool.tile([128, NQT], F32)
            cw = cpool.tile([128, NQT], F32)
            idxs = cpool.tile([128, NIDX16], I16)
            iob = cpool.tile([128, 128], BF16)
            dis = cpool.tile([128, NBLK], F32)
            nc.sync.dma_start(out=rl[:], in_=rowt[:, :])
            nc.sync.dma_start(out=cw[:], in_=ct[:, :])
            nc.sync.dma_start(out=idxs[:], in_=idxt[:, :])
            nc.sync.dma_start(out=iob[:], in_=iotab[:, :])
            nc.sync.dma_start(out=dis[:], in_=dis_sl[:, :])
            if use_act:
                nrl = cpool.tile([128, NQT], F32)
                ncw = cpool.tile([128, NQT], F32)
                nc.vector.tensor_scalar(out=nrl[:], in0=rl[:], scalar1=-1.0,
                                        scalar2=None, op0=mybir.AluOpType.mult)
                nc.vector.tensor_scalar(out=ncw[:], in0=cw[:], scalar1=-1.0,
                                        scalar2=None, op0=mybir.AluOpType.mult)
            if layer == 1:
                idb = cpool.tile([128, 128], BF16)
                b1s = cpool.tile([128, H], F32)
                b1b = cpool.tile([128, H], BF16)
                w2s = cpool.tile([H, C_PAD], F32)
                w2b = cpool.tile([H, C_PAD], BF16)
                nc.sync.dma_start(out=idb[:], in_=identb[:, :])
                nc.sync.dma_start(out=b1s[:], in_=b1bc[:, :])
                nc.sync.dma_start(out=w2s[:], in_=w2t[:, :])
                nc.vector.tensor_copy(out=b1b[:], in_=b1s[:])
                nc.vector.tensor_copy(out=w2b[:], in_=w2s[:])
            else:
                b2s = cpool.tile([128, C_PAD], F32)
                nc.sync.dma_start(out=b2s[:], in_=b2bc[:, :])

            rep = tc.For_i(0, reps, 1) if reps > 1 else contextlib.nullcontext()
            with rep:
                b0 = 0
                off16 = 0
                qn = 0
                ohi = 0
                BPB = 2048 // (4 * MM_W)   # blocks per PSUM bank
                for sbs in SUPERS:
                    gs = []
                    for k in range(NBUCK):
                        nidx = sbs * TBB * 128
                        g = gpool.tile([128, sbs * TBB * W_IN], BF16,
                                       tag=f"g{k}_{sbs}")
                        nc.gpsimd.dma_gather(
                            g[:].rearrange("p (t d) -> p t d", d=W_IN),
                            tab[k * BUCK:(k + 1) * BUCK, :],
                            idxs[:, off16:off16 + nidx // 16],
                            nidx, nidx, W_IN,
                            single_packet=False,
                            queue_num=qn % NSWQ,
                        )
                        gs.append(g)
                        off16 += nidx // 16
                        qn += 1
                    accs = {}
                    for bl in range(sbs):
                        bb = bl // BPB
                        if bb not in accs:
                            nacc = min(BPB, sbs - bb * BPB)
                            acc_t = apool.tile([128, nacc * MM_W], F32,
                                               tag=f"acc{bb}", name=f"acc{bb}")
                            accs[bb] = acc_t
                    for bl in range(sbs):
                        b = b0 + bl
                        aoff = (bl % BPB) * MM_W
                        acc = accs[bl // BPB][:, aoff:aoff + MM_W]
                        nmm = NBUCK * TBB
                        mi = 0
                        for k in range(NBUCK):
                            for t in range(TBB):
                                kk = (b * NBUCK + k) * TBB + t
                                oh = opool.tile([128, 128], BF16, tag="oh")
                                e = eng_seq[ohi % len(eng_seq)]
                                ohi += 1
                                if e == "v":
                                    nc.vector.tensor_scalar(
                                        out=oh[:], in0=iob[:],
                                        scalar1=rl[:, kk:kk + 1],
                                        scalar2=cw[:, kk:kk + 1],
                                        op0=mybir.AluOpType.is_equal,
                                        op1=mybir.AluOpType.mult)
                                elif e == "p":
                                    nc.gpsimd.tensor_scalar(
                                        out=oh[:], in0=iob[:],
                                        scalar1=rl[:, kk:kk + 1],
                                        scalar2=cw[:, kk:kk + 1],
                                        op0=mybir.AluOpType.is_equal,
                                        op1=mybir.AluOpType.mult)
                                else:
                                    d2 = opool.tile([128, 128], BF16, tag="d2")
                                    nc.scalar.activation(
                                        out=d2[:], in_=iob[:],
                                        func=mybir.ActivationFunctionType.Square,
                                        bias=nrl[:, kk:kk + 1], scale=1.0)
                                    nc.scalar.activation(
                                        out=oh[:], in_=d2[:],
                                        func=mybir.ActivationFunctionType.Relu,
                                        bias=cw[:, kk:kk + 1],
                                        scale=ncw[:, kk:kk + 1])
                                nc.tensor.matmul(
                                    out=acc[:], lhsT=oh[:],
                                    rhs=gs[k][:, (bl * TBB + t) * W_IN:
                                              (bl * TBB + t) * W_IN + MM_W],
                                    start=(mi == 0), stop=(mi == nmm - 1))
                                mi += 1
                        if layer == 1:
                            tsb = tpool.tile([128, H], BF16, tag="t")
                            nc.scalar.activation(
                                out=tsb[:], in_=acc[:],
                                func=mybir.ActivationFunctionType.Copy,
                                scale=dis[:, b:b + 1])
                            usb = tpool.tile([128, H], BF16, tag="u")
                            nc.vector.tensor_tensor(
                                out=usb[:], in0=tsb[:], in1=b1b[:],
                                op=mybir.AluOpType.add)
                            hsb = tpool.tile([128, H], BF16, tag="h")
                            nc.vector.tensor_scalar(
                                out=hsb[:], in0=usb[:], scalar1=0.0,
                                scalar2=None, op0=mybir.AluOpType.max)
                            hT_ps = ppool2.tile([128, 128], BF16, tag="hT")
                            nc.tensor.transpose(out=hT_ps[:], in_=hsb[:],
                                                identity=idb[:])
                            hT = tpool.tile([128, 128], BF16, tag="hTs")
                            nc.scalar.copy(out=hT[:], in_=hT_ps[:])
                            hw2_ps = ppool2.tile([128, C_PAD], F32, tag="hw2")
                            nc.tensor.matmul(out=hw2_ps[:], lhsT=hT[:],
                                             rhs=w2b[:], start=True, stop=True)
                            osb = tpool.tile([128, C_PAD], BF16, tag="o")
                            nc.scalar.activation(
                                out=osb[:], in_=hw2_ps[:],
                                func=mybir.ActivationFunctionType.Copy,
                                scale=dis[:, b:b + 1])
                            nc.sync.dma_start(
                                out=out_sl[b * 128:(b + 1) * 128, :C_PAD],
                                in_=osb[:])
                        else:
                            o1 = tpool.tile([128, C_PAD], F32, tag="o1")
                            nc.scalar.activation(
                                out=o1[:], in_=acc[:],
                                func=mybir.ActivationFunctionType.Copy,
                                scale=dis[:, b:b + 1])
                            o2 = tpool.tile([128, C_PAD], F32, tag="o2")
                            nc.vector.tensor_tensor(
                                out=o2[:], in0=o1[:], in1=b2s[:],
                                op=mybir.AluOpType.add)
                            nc.sync.dma_start(
                                out=out_sl[b * 128:(b + 1) * 128, :],
                                in_=o2[:])
                    b0 += sbs
    nc.compile()
    return nc


_CACHE = {}


def _make_in_maps(x, W1, b1v, W2, b2v, L):
    iota_bf, ident_bf = make_consts()
    x_pad = np.zeros((NPAD, D), np.float32)
    x_pad[:N] = x
    cores = list(range(NCORES))
    in1 = [{"x_sl": x_pad[c * RPC:(c + 1) * RPC], "cvrow": L["cvrow"][c],
            "w1t": np.ascontiguousarray(W1.T), "identb": ident_bf}
           for c in cores]
    b1bc = np.broadcast_to(b1v, (128, H)).copy()
    w2t = np.zeros((H, C_PAD), np.float32)
    w2t[:, :C_OUT] = W2.T
    in2_fixed = [{"rowt": L["rl"][c], "ct": L["cw"][c], "idxt": L["idxw"][c],
                  "iotab": iota_bf, "identb": ident_bf, "b1bc": b1bc,
                  "w2t": w2t} for c in cores]
    b2bc = np.zeros((128, C_PAD), np.float32)
    b2bc[:, :C_OUT] = b2v
    in3_fixed = [{"rowt": L["rl"][c], "ct": L["cw"][c], "idxt": L["idxw"][c],
                  "iotab": iota_bf, "b2bc": b2bc} for c in cores]
    return in1, in2_fixed, in3_fixed


def kernel(x, edge_index, C_values, W1, b1, W2, b2):
    x = np.asarray(x, np.float32)
    row = np.asarray(edge_index[0], np.int64)
    col = np.asarray(edge_index[1], np.int64)
    cv = np.asarray(C_values, np.float32)
    W1 = np.asarray(W1, np.float32)
    b1v = np.asarray(b1, np.float32)
    W2 = np.asarray(W2, np.float32)
    b2v = np.asarray(b2, np.float32)

    L = build_host_layouts(row, col, cv)
    key = (L["k1"], L["tbb"])
    if key not in _CACHE:
        _CACHE[key] = (build_launch1(L["k1"]), build_spmm(L["tbb"], 1),
                       build_spmm(L["tbb"], 2))
    nc1, nc2, nc3 = _CACHE[key]

    in1, in2f, in3f = _make_in_maps(x, W1, b1v, W2, b2v, L)
    cores = list(range(NCORES))

    r1 = run_bass_kernel_spmd(nc1, in1, core_ids=cores, trace=False)
    xw1_full = np.concatenate([r1.results[c]["xw1_sl"] for c in cores], axis=0)
    dis = [r1.results[c]["dis_sl"] for c in cores]

    in2 = [{**in2f[c], "tab": xw1_full, "dis_sl": dis[c]} for c in cores]
    r2 = run_bass_kernel_spmd(nc2, in2, core_ids=cores, trace=False)
    hw2_full = np.concatenate([r2.results[c]["out_sl"] for c in cores], axis=0)

    in3 = [{**in3f[c], "tab": hw2_full, "dis_sl": dis[c]} for c in cores]
    r3 = run_bass_kernel_spmd(nc3, in3, core_ids=cores, trace=False)
    out = np.concatenate([r3.results[c]["out_sl"] for c in cores], axis=0)
    return np.ascontiguousarray(out[:N, :C_OUT]).astype(np.float32)


def bench_launches(x, edge_index, C_values, W1, b1, W2, b2, reps, measure):
    """Reps-diff device timing per launch via `measure(nc, in_maps, cores)`."""
    x = np.asarray(x, np.float32)
    row = np.asarray(edge_index[0], np.int64)
    col = np.asarray(edge_index[1], np.int64)
    cv = np.asarray(C_values, np.float32)
    W1 = np.asarray(W1, np.float32)
    b1v = np.asarray(b1, np.float32)
    W2 = np.asarray(W2, np.float32)
    b2v = np.asarray(b2, np.float32)

    L = build_host_layouts(row, col, cv)
    in1, in2f, in3f = _make_in_maps(x, W1, b1v, W2, b2v, L)
    cores = list(range(NCORES))

    nc1 = build_launch1(L["k1"])
    r1 = run_bass_kernel_spmd(nc1, in1, core_ids=cores, trace=False)
    xw1_full = np.concatenate([r1.results[c]["xw1_sl"] for c in cores], axis=0)
    dis = [r1.results[c]["dis_sl"] for c in cores]
    in2 = [{**in2f[c], "tab": xw1_full, "dis_sl": dis[c]} for c in cores]
    nc2 = build_spmm(L["tbb"], 1)
    r2 = run_bass_kernel_spmd(nc2, in2, core_ids=cores, trace=False)
    hw2_full = np.concatenate([r2.results[c]["out_sl"] for c in cores], axis=0)
    in3 = [{**in3f[c], "tab": hw2_full, "dis_sl": dis[c]} for c in cores]
    nc3 = build_spmm(L["tbb"], 2)

    times = {}
    for name, nc_a, build_r, im in [
        ("launch1", nc1, lambda: build_launch1(L["k1"], reps=reps), in1),
        ("launch2", nc2, lambda: build_spmm(L["tbb"], 1, reps=reps), in2),
        ("launch3", nc3, lambda: build_spmm(L["tbb"], 2, reps=reps), in3),
    ]:
        t1 = measure(nc_a, im, cores)
        tR = measure(build_r(), im, cores)
        times[name] = max(tR - t1, 0.0) / (reps - 1)
        print(f"{name}: t1={t1:.3f}s tR={tR:.3f}s -> "
              f"{times[name]*1e9:.0f} ns/rep", flush=True)
    return times


# revision 8
# speedup vs baseline: 8756.9833x; 1.0117x over previous
"""Trainium2 Bass kernel for the 2-layer GCN (nn_CGNN_70566312673786).

Strategy (8 NeuronCores, SPMD, 3 NEFF launches with host-side concat of
per-core slices between them — index prep / data movement only on host,
all float math on device):

  L1: deg = segment_sum(cv, row) computed WITHOUT one-hots: the host slots
      each edge's cv into a [128, NBLK, K1] per-dest-row layout and one DVE
      tensor_reduce produces deg for all rows; dis = rsqrt-guarded.
      xw1 = (dis * x) @ W1^T is emitted bf16 (spmm and the linear commute).
  L2: per-edge source rows xw1[col] are fetched with supergathers —
      one dma_gather per (8-block super, int16 col bucket), 4 SWDGE
      queues, >=8 gather buffers in flight (descriptor-rate bound
      otherwise).  segment_sum runs as one-hot matmuls on the PE in bf16:
      oh[e,r] = cv_e * (iota[r]==rl_e) built on DVE (fused
      is_equal+mult) / Pool / ACT (relu(cv - cv*(iota-rl)^2)) round-robin.
      Tail per 128-row block: h = relu(dis*acc + b1); hw2s = dis*(h@W2^T)
      written as [RPC, 128] bf16 rows (cols 0:64; 256B gather rows).
  L3: same supergather/spmm structure over hw2s; out = dis*acc + b2.
"""

import contextlib

import numpy as np

import concourse.bacc as bacc
import concourse.bass as bass
import concourse.mybir as mybir
import concourse.tile as tile
from concourse.bass_utils import run_bass_kernel_spmd

N = 100000
E = 1600000
D = 128
H = 128
C_OUT = 40
C_PAD = 64         # hw2 compute width (psum free dim)

NCORES = 8
RPC = 12544
NPAD = NCORES * RPC
NBLK = RPC // 128  # 98
NBUCK = 4
BUCK = NPAD // NBUCK  # 25088

SB = 7             # blocks per supergather
SUPERS = [SB] * (NBLK // SB) + ([NBLK % SB] if NBLK % SB else [])
NSWQ = 4           # swdge queues
GBUFS = 3          # gather buffers per (bucket,size) tag

F32 = mybir.dt.float32
BF16 = mybir.dt.bfloat16
I16 = mybir.dt.int16


def _np_bf16():
    import ml_dtypes
    return np.dtype(ml_dtypes.bfloat16)


def _wrap_idx(flat):
    """dma_gather idx layout: idx i -> [i % 16, i // 16] x8 replication."""
    n = flat.shape[0]
    w = flat.reshape(n // 16, 16).T
    return np.tile(w, (8, 1))


def build_host_layouts(row, col, cv):
    """Vectorized host index prep (no float math: slotting/sorting only)."""
    row = row.astype(np.int64)
    col = col.astype(np.int64)
    cv = cv.astype(np.float32)

    # deg layout: slot each edge per dest row
    order_r = np.argsort(row, kind="stable")
    rs = row[order_r]
    cvs = cv[order_r]
    starts = np.searchsorted(rs, np.arange(NPAD))
    slot = np.arange(E) - starts[rs]
    K1 = int(slot.max()) + 1
    cvrow = np.zeros((NCORES, 128, NBLK * K1), np.float32)
    cvrow[rs // RPC, rs % 128, ((rs % RPC) // 128) * K1 + slot] = cvs
    assert (np.bincount(row, minlength=N)[:N] > 0).all(), \
        "zero-in-degree row: rank-1 bias-fold in build_spmm would be wrong"

    # spmm layout: group edges by (dest 128-block, col bucket)
    gblk = row // 128
    key = gblk * NBUCK + (col // BUCK)
    order = np.argsort(key, kind="stable")
    key_s = key[order]
    col_s = col[order]
    row_s = row[order]
    cv_s = cv[order]
    NG = NCORES * NBLK * NBUCK
    gstarts = np.searchsorted(key_s, np.arange(NG))
    gends = np.searchsorted(key_s, np.arange(NG), side="right")
    cnts = gends - gstarts
    TBB = int(np.ceil(cnts.max() / 128))
    NQ = NBLK * NBUCK

    pos = np.arange(E) - gstarts[key_s]
    gq = key_s % NQ
    gcore = key_s // NQ
    rl_flat = np.zeros((NCORES, NQ, TBB * 128), np.float32)
    cw_flat = np.zeros((NCORES, NQ, TBB * 128), np.float32)
    ix_flat = np.zeros((NCORES, NQ, TBB * 128), np.int16)  # 0-pads (see DGE)
    rl_flat[gcore, gq, pos] = (row_s % 128).astype(np.float32)
    cw_flat[gcore, gq, pos] = cv_s
    ix_flat[gcore, gq, pos] = (col_s % BUCK).astype(np.int16)

    rl_t = np.ascontiguousarray(
        rl_flat.reshape(NCORES, NQ * TBB, 128).transpose(0, 2, 1))
    cw_t = np.ascontiguousarray(
        cw_flat.reshape(NCORES, NQ * TBB, 128).transpose(0, 2, 1))

    # supergather idx streams ordered (super, bucket, block)
    idxw = []
    for c in range(NCORES):
        streams = []
        b0 = 0
        for sbs in SUPERS:
            for k in range(NBUCK):
                qs = [(b * NBUCK + k) for b in range(b0, b0 + sbs)]
                streams.append(ix_flat[c, qs].reshape(-1))
            b0 += sbs
        idxw.append(np.concatenate([_wrap_idx(s) for s in streams], axis=1))
    idxw = np.stack(idxw)

    return dict(k1=K1, cvrow=cvrow, rl=rl_t, cw=cw_t, idxw=idxw, tbb=TBB)


def make_consts():
    bf = _np_bf16()
    iota = np.tile(np.arange(128).astype(bf), (128, 1))
    ident = np.eye(128).astype(bf)
    return iota, ident


def build_launch1(K1, reps=1):
    nc = bacc.Bacc("TRN2", target_bir_lowering=False)
    x_sl = nc.dram_tensor("x_sl", [RPC, D], F32, kind="ExternalInput")
    cvrow = nc.dram_tensor("cvrow", [128, NBLK * K1], F32, kind="ExternalInput")
    w1t = nc.dram_tensor("w1t", [D, H], F32, kind="ExternalInput")
    identb = nc.dram_tensor("identb", [128, 128], BF16, kind="ExternalInput")
    xw1_sl = nc.dram_tensor("xw1_sl", [RPC, H], BF16, kind="ExternalOutput")
    dis_sl = nc.dram_tensor("dis_sl", [128, NBLK], F32, kind="ExternalOutput")
    invd_sl = nc.dram_tensor("invd_sl", [128, NBLK], BF16, kind="ExternalOutput")

    with tile.TileContext(nc) as tc:
        with tc.tile_pool(name="const", bufs=1) as cpool, \
             tc.tile_pool(name="work", bufs=3) as wpool, \
             tc.tile_pool(name="psum", bufs=2, space="PSUM") as ppool, \
             tc.tile_pool(name="psum2", bufs=2, space="PSUM") as ppool2:
            cvr = cpool.tile([128, NBLK * K1], F32)
            w1s = cpool.tile([D, H], F32)
            w1b = cpool.tile([D, H], BF16)
            idb = cpool.tile([128, 128], BF16)
            deg = cpool.tile([128, NBLK], F32)
            z = cpool.tile([128, NBLK], F32)
            sq = cpool.tile([128, NBLK], F32)
            rec = cpool.tile([128, NBLK], F32)
            dis = cpool.tile([128, NBLK], F32)
            nc.sync.dma_start(out=cvr[:], in_=cvrow[:, :])
            nc.sync.dma_start(out=w1s[:], in_=w1t[:, :])
            nc.sync.dma_start(out=idb[:], in_=identb[:, :])
            nc.vector.tensor_copy(out=w1b[:], in_=w1s[:])

            rep = tc.For_i(0, reps, 1) if reps > 1 else contextlib.nullcontext()
            with rep:
                nc.vector.tensor_reduce(
                    out=deg[:], in_=cvr[:].rearrange("p (b k) -> p b k", k=K1),
                    axis=mybir.AxisListType.X, op=mybir.AluOpType.add)
                nc.vector.tensor_scalar(
                    out=z[:], in0=deg[:], scalar1=0.0, scalar2=None,
                    op0=mybir.AluOpType.is_le)
                nc.vector.tensor_tensor(
                    out=deg[:], in0=deg[:], in1=z[:], op=mybir.AluOpType.add)
                nc.scalar.sqrt(out=sq[:], in_=deg[:])
                nc.vector.reciprocal(out=rec[:], in_=sq[:])
                nc.vector.tensor_scalar(
                    out=z[:], in0=z[:], scalar1=-1.0, scalar2=1.0,
                    op0=mybir.AluOpType.mult, op1=mybir.AluOpType.add)
                nc.vector.tensor_tensor(
                    out=dis[:], in0=rec[:], in1=z[:], op=mybir.AluOpType.mult)
                nc.sync.dma_start(out=dis_sl[:, :], in_=dis[:])
                sqb = cpool.tile([128, NBLK], BF16, name="sqb")
                nc.vector.tensor_copy(out=sqb[:], in_=sq[:])
                nc.sync.dma_start(out=invd_sl[:, :], in_=sqb[:])

                for b in range(NBLK):
                    xt = wpool.tile([128, D], F32, tag="xt")
                    nc.sync.dma_start(out=xt[:],
                                      in_=x_sl[b * 128:(b + 1) * 128, :])
                    xb = wpool.tile([128, D], BF16, tag="xb")
                    nc.vector.tensor_scalar(
                        out=xb[:], in0=xt[:], scalar1=dis[:, b:b + 1],
                        scalar2=None, op0=mybir.AluOpType.mult)
                    xbT_ps = ppool.tile([128, 128], BF16, tag="xbT")
                    nc.tensor.transpose(out=xbT_ps[:], in_=xb[:],
                                        identity=idb[:])
                    xbT = wpool.tile([128, 128], BF16, tag="xbTs")
                    nc.scalar.copy(out=xbT[:], in_=xbT_ps[:])
                    xw1_ps = ppool2.tile([128, H], F32, tag="xw1")
                    nc.tensor.matmul(out=xw1_ps[:], lhsT=xbT[:], rhs=w1b[:],
                                     start=True, stop=True)
                    xw1b = wpool.tile([128, H], BF16, tag="xw1b")
                    nc.scalar.copy(out=xw1b[:], in_=xw1_ps[:])
                    nc.sync.dma_start(out=xw1_sl[b * 128:(b + 1) * 128, :],
                                      in_=xw1b[:])
    nc.compile()
    return nc


def build_spmm(TBB, layer, reps=1,
               onehot_plan=(("v", 1),)):
    NQ = NBLK * NBUCK
    NQT = NQ * TBB
    NIDX16 = NQT * 8
    W_IN = 128
    MM_W = H if layer == 1 else C_PAD
    nc = bacc.Bacc("TRN2", target_bir_lowering=False, num_swdge_queues=NSWQ)
    tab = nc.dram_tensor("tab", [NPAD, W_IN], BF16, kind="ExternalInput")
    rowt = nc.dram_tensor("rowt", [128, NQT], F32, kind="ExternalInput")
    ct = nc.dram_tensor("ct", [128, NQT], F32, kind="ExternalInput")
    idxt = nc.dram_tensor("idxt", [128, NIDX16], I16, kind="ExternalInput")
    iotab = nc.dram_tensor("iotab", [128, 128], BF16, kind="ExternalInput")
    dis_sl = nc.dram_tensor("dis_sl", [128, NBLK], F32, kind="ExternalInput")
    invdT_d = nc.dram_tensor("invdT", [1, NBLK * 128], BF16,
                             kind="ExternalInput")
    identb = nc.dram_tensor("identb", [128, 128], BF16, kind="ExternalInput")
    if layer == 1:
        b1bc = nc.dram_tensor("b1bc", [128, H], F32, kind="ExternalInput")
        w2t = nc.dram_tensor("w2t", [H, C_PAD], F32, kind="ExternalInput")
        out_sl = nc.dram_tensor("out_sl", [RPC, W_IN], BF16,
                                kind="ExternalOutput")
    else:
        b2bc = nc.dram_tensor("b2bc", [128, C_PAD], F32, kind="ExternalInput")
        out_sl = nc.dram_tensor("out_sl", [RPC, C_PAD], F32,
                                kind="ExternalOutput")

    eng_seq = []
    for e, w in onehot_plan:
        eng_seq += [e] * w

    with tile.TileContext(nc) as tc:
        with tc.tile_pool(name="const", bufs=1) as cpool, \
             tc.tile_pool(name="gat", bufs=GBUFS) as gpool, \
             tc.tile_pool(name="oh", bufs=12) as opool, \
             tc.tile_pool(name="tail", bufs=3) as tpool, \
             tc.tile_pool(name="acc", bufs=2, space="PSUM") as apool, \
             tc.tile_pool(name="ps2", bufs=2, space="PSUM") as ppool2:
            use_act = any(e == "a" for e, _ in onehot_plan)
            rl = cpool.tile([128, NQT], F32)
            cw = cpool.tile([128, NQT], F32)
            idxs = cpool.tile([128, NIDX16], I16)
            iob = cpool.tile([128, 128], BF16)
            dis = cpool.tile([128, NBLK], F32)
            nc.sync.dma_start(out=rl[:], in_=rowt[:, :])
            nc.sync.dma_start(out=cw[:], in_=ct[:, :])
            nc.sync.dma_start(out=idxs[:], in_=idxt[:, :])
            nc.sync.dma_start(out=iob[:], in_=iotab[:, :])
            nc.sync.dma_start(out=dis[:], in_=dis_sl[:, :])
            ivT = cpool.tile([1, NBLK * 128], BF16)
            idb = cpool.tile([128, 128], BF16)
            nc.sync.dma_start(out=ivT[:], in_=invdT_d[:, :])
            nc.sync.dma_start(out=idb[:], in_=identb[:, :])
            if use_act:
                nrl = cpool.tile([128, NQT], F32)
                ncw = cpool.tile([128, NQT], F32)
                nc.vector.tensor_scalar(out=nrl[:], in0=rl[:], scalar1=-1.0,
                                        scalar2=None, op0=mybir.AluOpType.mult)
                nc.vector.tensor_scalar(out=ncw[:], in0=cw[:], scalar1=-1.0,
                                        scalar2=None, op0=mybir.AluOpType.mult)
            if layer == 1:
                b1s = cpool.tile([128, H], F32)
                b1b = cpool.tile([128, H], BF16)
                w2s = cpool.tile([H, C_PAD], F32)
                w2b = cpool.tile([H, C_PAD], BF16)
                nc.sync.dma_start(out=b1s[:], in_=b1bc[:, :])
                nc.sync.dma_start(out=w2s[:], in_=w2t[:, :])
                nc.vector.tensor_copy(out=b1b[:], in_=b1s[:])
                nc.vector.tensor_copy(out=w2b[:], in_=w2s[:])
                brow = b1b
            else:
                b2s = cpool.tile([128, C_PAD], F32)
                b2b = cpool.tile([128, C_PAD], BF16)
                nc.sync.dma_start(out=b2s[:], in_=b2bc[:, :])
                nc.vector.tensor_copy(out=b2b[:], in_=b2s[:])
                brow = b2b

            rep = tc.For_i(0, reps, 1) if reps > 1 else contextlib.nullcontext()
            with rep:
                b0 = 0
                off16 = 0
                qn = 0
                ohi = 0
                BPB = 2048 // (4 * MM_W)   # blocks per PSUM bank
                for sbs in SUPERS:
                    gs = []
                    for k in range(NBUCK):
                        nidx = sbs * TBB * 128
                        g = gpool.tile([128, sbs * TBB * W_IN], BF16,
                                       tag=f"g{k}_{sbs}")
                        nc.gpsimd.dma_gather(
                            g[:].rearrange("p (t d) -> p t d", d=W_IN),
                            tab[k * BUCK:(k + 1) * BUCK, :],
                            idxs[:, off16:off16 + nidx // 16],
                            nidx, nidx, W_IN,
                            single_packet=False,
                            queue_num=qn % NSWQ,
                        )
                        gs.append(g)
                        off16 += nidx // 16
                        qn += 1
                    accs = {}
                    for bl in range(sbs):
                        bb = bl // BPB
                        if bb not in accs:
                            nacc = min(BPB, sbs - bb * BPB)
                            acc_t = apool.tile([128, nacc * MM_W], F32,
                                               tag=f"acc{bb}", name=f"acc{bb}")
                            accs[bb] = acc_t
                    for bl in range(sbs):
                        b = b0 + bl
                        aoff = (bl % BPB) * MM_W
                        acc = accs[bl // BPB][:, aoff:aoff + MM_W]
                        nc.tensor.matmul(
                            out=acc[:], lhsT=ivT[0:1, b * 128:(b + 1) * 128],
                            rhs=brow[0:1, :MM_W], start=True, stop=False)
                        nmm = NBUCK * TBB
                        mi = 1
                        for k in range(NBUCK):
                            for t in range(TBB):
                                kk = (b * NBUCK + k) * TBB + t
                                oh = opool.tile([128, 128], BF16, tag="oh")
                                e = eng_seq[ohi % len(eng_seq)]
                                ohi += 1
                                if e == "v":
                                    nc.vector.tensor_scalar(
                                        out=oh[:], in0=iob[:],
                                        scalar1=rl[:, kk:kk + 1],
                                        scalar2=cw[:, kk:kk + 1],
                                        op0=mybir.AluOpType.is_equal,
                                        op1=mybir.AluOpType.mult)
                                elif e == "p":
                                    nc.gpsimd.tensor_scalar(
                                        out=oh[:], in0=iob[:],
                                        scalar1=rl[:, kk:kk + 1],
                                        scalar2=cw[:, kk:kk + 1],
                                        op0=mybir.AluOpType.is_equal,
                                        op1=mybir.AluOpType.mult)
                                else:
                                    d2 = opool.tile([128, 128], BF16, tag="d2")
                                    nc.scalar.activation(
                                        out=d2[:], in_=iob[:],
                                        func=mybir.ActivationFunctionType.Square,
                                        bias=nrl[:, kk:kk + 1], scale=1.0)
                                    nc.scalar.activation(
                                        out=oh[:], in_=d2[:],
                                        func=mybir.ActivationFunctionType.Relu,
                                        bias=cw[:, kk:kk + 1],
                                        scale=ncw[:, kk:kk + 1])
                                nc.tensor.matmul(
                                    out=acc[:], lhsT=oh[:],
                                    rhs=gs[k][:, (bl * TBB + t) * W_IN:
                                              (bl * TBB + t) * W_IN + MM_W],
                                    start=False, stop=(mi == nmm))
                                mi += 1
                        if layer == 1:
                            hsb = tpool.tile([128, H], BF16, tag="h")
                            nc.scalar.activation(
                                out=hsb[:], in_=acc[:],
                                func=mybir.ActivationFunctionType.Relu,
                                scale=dis[:, b:b + 1])
                            hT_ps = ppool2.tile([128, 128], BF16, tag="hT")
                            nc.tensor.transpose(out=hT_ps[:], in_=hsb[:],
                                                identity=idb[:])
                            hT = tpool.tile([128, 128], BF16, tag="hTs")
                            nc.scalar.copy(out=hT[:], in_=hT_ps[:])
                            hw2_ps = ppool2.tile([128, C_PAD], F32, tag="hw2")
                            nc.tensor.matmul(out=hw2_ps[:], lhsT=hT[:],
                                             rhs=w2b[:], start=True, stop=True)
                            osb = tpool.tile([128, C_PAD], BF16, tag="o")
                            nc.scalar.activation(
                                out=osb[:], in_=hw2_ps[:],
                                func=mybir.ActivationFunctionType.Copy,
                                scale=dis[:, b:b + 1])
                            nc.sync.dma_start(
                                out=out_sl[b * 128:(b + 1) * 128, :C_PAD],
                                in_=osb[:])
                        else:
                            o1 = tpool.tile([128, C_PAD], F32, tag="o1")
                            nc.scalar.activation(
                                out=o1[:], in_=acc[:],
                                func=mybir.ActivationFunctionType.Copy,
                                scale=dis[:, b:b + 1])
                            nc.sync.dma_start(
                                out=out_sl[b * 128:(b + 1) * 128, :],
                                in_=o1[:])
                    b0 += sbs
    nc.compile()
    return nc


_CACHE = {}


def _make_in_maps(x, W1, b1v, W2, b2v, L):
    iota_bf, ident_bf = make_consts()
    x_pad = np.zeros((NPAD, D), np.float32)
    x_pad[:N] = x
    cores = list(range(NCORES))
    in1 = [{"x_sl": x_pad[c * RPC:(c + 1) * RPC], "cvrow": L["cvrow"][c],
            "w1t": np.ascontiguousarray(W1.T), "identb": ident_bf}
           for c in cores]
    b1bc = np.broadcast_to(b1v, (128, H)).copy()
    w2t = np.zeros((H, C_PAD), np.float32)
    w2t[:, :C_OUT] = W2.T
    in2_fixed = [{"rowt": L["rl"][c], "ct": L["cw"][c], "idxt": L["idxw"][c],
                  "iotab": iota_bf, "identb": ident_bf, "b1bc": b1bc,
                  "w2t": w2t} for c in cores]
    b2bc = np.zeros((128, C_PAD), np.float32)
    b2bc[:, :C_OUT] = b2v
    in3_fixed = [{"rowt": L["rl"][c], "ct": L["cw"][c], "idxt": L["idxw"][c],
                  "iotab": iota_bf, "identb": ident_bf, "b2bc": b2bc}
                 for c in cores]
    return in1, in2_fixed, in3_fixed


def kernel(x, edge_index, C_values, W1, b1, W2, b2):
    x = np.asarray(x, np.float32)
    row = np.asarray(edge_index[0], np.int64)
    col = np.asarray(edge_index[1], np.int64)
    cv = np.asarray(C_values, np.float32)
    W1 = np.asarray(W1, np.float32)
    b1v = np.asarray(b1, np.float32)
    W2 = np.asarray(W2, np.float32)
    b2v = np.asarray(b2, np.float32)

    L = build_host_layouts(row, col, cv)
    key = (L["k1"], L["tbb"])
    if key not in _CACHE:
        _CACHE[key] = (build_launch1(L["k1"]), build_spmm(L["tbb"], 1),
                       build_spmm(L["tbb"], 2))
    nc1, nc2, nc3 = _CACHE[key]

    in1, in2f, in3f = _make_in_maps(x, W1, b1v, W2, b2v, L)
    cores = list(range(NCORES))

    r1 = run_bass_kernel_spmd(nc1, in1, core_ids=cores, trace=False)
    xw1_full = np.concatenate([r1.results[c]["xw1_sl"] for c in cores], axis=0)
    dis = [r1.results[c]["dis_sl"] for c in cores]
    invd = [r1.results[c]["invd_sl"] for c in cores]

    ivT = [np.ascontiguousarray(invd[c].T) for c in cores]
    in2 = [{**in2f[c], "tab": xw1_full, "dis_sl": dis[c],
            "invdT": ivT[c].reshape(1, -1)} for c in cores]
    r2 = run_bass_kernel_spmd(nc2, in2, core_ids=cores, trace=False)
    hw2_full = np.concatenate([r2.results[c]["out_sl"] for c in cores], axis=0)

    in3 = [{**in3f[c], "tab": hw2_full, "dis_sl": dis[c],
            "invdT": ivT[c].reshape(1, -1)} for c in cores]
    r3 = run_bass_kernel_spmd(nc3, in3, core_ids=cores, trace=False)
    out = np.concatenate([r3.results[c]["out_sl"] for c in cores], axis=0)
    return np.ascontiguousarray(out[:N, :C_OUT]).astype(np.float32)


def bench_launches(x, edge_index, C_values, W1, b1, W2, b2, reps, measure):
    """Reps-diff device timing per launch via `measure(nc, in_maps, cores)`."""
    x = np.asarray(x, np.float32)
    row = np.asarray(edge_index[0], np.int64)
    col = np.asarray(edge_index[1], np.int64)
    cv = np.asarray(C_values, np.float32)
    W1 = np.asarray(W1, np.float32)
    b1v = np.asarray(b1, np.float32)
    W2 = np.asarray(W2, np.float32)
    b2v = np.asarray(b2, np.float32)

    L = build_host_layouts(row, col, cv)
    in1, in2f, in3f = _make_in_maps(x, W1, b1v, W2, b2v, L)
    cores = list(range(NCORES))

    nc1 = build_launch1(L["k1"])
    r1 = run_bass_kernel_spmd(nc1, in1, core_ids=cores, trace=False)
    xw1_full = np.concatenate([r1.results[c]["xw1_sl"] for c in cores], axis=0)
    dis = [r1.results[c]["dis_sl"] for c in cores]
    invd = [r1.results[c]["invd_sl"] for c in cores]
    ivT = [np.ascontiguousarray(invd[c].T) for c in cores]
    in2 = [{**in2f[c], "tab": xw1_full, "dis_sl": dis[c],
            "invdT": ivT[c].reshape(1, -1)} for c in cores]
    nc2 = build_spmm(L["tbb"], 1)
    r2 = run_bass_kernel_spmd(nc2, in2, core_ids=cores, trace=False)
    hw2_full = np.concatenate([r2.results[c]["out_sl"] for c in cores], axis=0)
    in3 = [{**in3f[c], "tab": hw2_full, "dis_sl": dis[c],
            "invdT": ivT[c].reshape(1, -1)} for c in cores]
    nc3 = build_spmm(L["tbb"], 2)

    times = {}
    for name, nc_a, build_r, im in [
        ("launch1", nc1, lambda: build_launch1(L["k1"], reps=reps), in1),
        ("launch2", nc2, lambda: build_spmm(L["tbb"], 1, reps=reps), in2),
        ("launch3", nc3, lambda: build_spmm(L["tbb"], 2, reps=reps), in3),
    ]:
        t1 = measure(nc_a, im, cores)
        tR = measure(build_r(), im, cores)
        times[name] = max(tR - t1, 0.0) / (reps - 1)
        print(f"{name}: t1={t1:.3f}s tR={tR:.3f}s -> "
              f"{times[name]*1e9:.0f} ns/rep", flush=True)
    return times
